# revision 9
# baseline (speedup 1.0000x reference)
"""Trainium2 Bass kernel for the 3-layer spiking neural network (DSNN).

Strategy
--------
Data-parallel over batch: 256 rows / 8 cores = 32 per core, weights
replicated, zero collectives. Inside each core the computation is
restructured so the PE does large timestep-batched matmuls while only the
cheap elementwise membrane dynamics stay sequential:

  1. Spike train S (binary) is generated on-device (DVE compare) in
     feature-major layout [512 feats, 99*32 (t,b)]. The bias feature always
     spikes (uniform r < 1), so its weight row is folded in as a bias add.
  2. H0 = S @ W0 for a block of timesteps at once (feature-major out).
  3. Layer-0 membrane recurrence: elementwise over t (DVE), emitting spike
     tiles s0.
  4. H1 = s0 @ W1 per block (feature-major).
  5. Layer-1 recurrence: syn/mem/spike/reset, plus accumulation of
     A = sum_t w_t * out1(t), where w_t is the closed-form alpha/beta decay
     weight. Layer 2 has no reset, so mem2 = A @ W2 exactly.
  6. Final small matmul A @ W2 -> output [32, 512].

Precision: matmuls run as float32r (FP22, 1 cyc/row — 4x faster than fp32).
W0 is fed as an exact hi+lo FP22 split (2 passes, recovering full fp32
precision where it matters most); W1/W2 are FP22-rounded single pass.
Measured end-to-end rel-l2 error vs the fp32 reference: ~9e-4 (the pure
fp32 noise floor from summation-order spike flips is ~2e-4).

Membrane reset trick: there is no reverse-subtract ALU op, so the stored
membrane is NEGATED: reset computes negm = (s - 1) * m' (in {-m', 0}) and
the next step uses scalar -beta: m' = (negm * -beta) + h.
"""

import numpy as np

ALPHA = 0.9
BETA = 0.85
THR = 1.0
T = 99            # timesteps actually simulated (t = 1..99 of 100)
BCORE = 32        # batch per core
NCORES = 8
TB = 16           # timesteps per pipeline block
BLOCK_SIZES = [TB] * (T // TB) + ([T % TB] if T % TB else [])

_CACHE = {}


def _round_m11(x):
    # hw float32r = e8m11, round-to-nearest on the 12 dropped bits
    # (empirically verified on trn2 via identity-matmul probe)
    xi = np.ascontiguousarray(np.asarray(x, np.float32)).view(np.uint32).astype(np.uint64)
    bias = np.uint64(0x7FF) + ((xi >> np.uint64(12)) & np.uint64(1))
    return ((xi + bias) & np.uint64(0xFFFFF000)).astype(np.uint32).view(np.float32)


def _decay_weights():
    # w_j = sum_{k=0}^{T-1-j} BETA^(T-1-j-k) * ALPHA^k
    w = np.zeros(T, np.float64)
    for j in range(T):
        n = T - 1 - j
        k = np.arange(n + 1)
        w[j] = np.sum(BETA ** (n - k) * (ALPHA ** k))
    return w.astype(np.float32)


def build_program():
    """Build + compile the (SPMD, per-core) Bass program once."""
    if "nc" in _CACHE:
        return _CACHE["nc"]
    import concourse.bacc as bacc
    import concourse.mybir as mybir
    import concourse.tile as tile

    f32 = mybir.dt.float32
    f32r = mybir.dt.float32r
    A = mybir.AluOpType
    Act = mybir.ActivationFunctionType

    W = _decay_weights()

    nc = bacc.Bacc("TRN2", target_bir_lowering=False, debug=False,
                   enable_asserts=False, num_devices=NCORES)

    RT = nc.dram_tensor("RT", [512, T * BCORE], f32, kind="ExternalInput").ap()
    xT = nc.dram_tensor("xT", [512, BCORE], f32, kind="ExternalInput").ap()
    W0h = nc.dram_tensor("W0h", [512, 1024], f32r, kind="ExternalInput").ap()
    W0l = nc.dram_tensor("W0l", [512, 1024], f32r, kind="ExternalInput").ap()
    W1d = nc.dram_tensor("W1d", [1024, 1024], f32r, kind="ExternalInput").ap()
    W2d = nc.dram_tensor("W2d", [1024, 512], f32r, kind="ExternalInput").ap()
    b0d = nc.dram_tensor("b0d", [128, 8], f32, kind="ExternalInput").ap()
    outd = nc.dram_tensor("out", [BCORE, 512], f32, kind="ExternalOutput").ap()

    with tile.TileContext(nc) as tc:
        with (
            tc.tile_pool(name="const", bufs=1) as cpool,
            tc.tile_pool(name="rt", bufs=2) as rt_pool,
            tc.tile_pool(name="sblk", bufs=2) as s_pool,
            tc.tile_pool(name="h0", bufs=2) as h0_pool,
            tc.tile_pool(name="h1", bufs=2) as h1_pool,
            tc.tile_pool(name="s0", bufs=1) as s0_pool,
            tc.tile_pool(name="tmp", bufs=2) as tmp_pool,
            tc.tile_pool(name="ps", bufs=8, space="PSUM") as ps_pool,
        ):
            # ---- constants ----
            w0h_sb = cpool.tile([128, 4 * 1024], f32r, tag="w0h")
            w0l_sb = cpool.tile([128, 4 * 1024], f32r, tag="w0l")
            w1_sb = cpool.tile([128, 8 * 1024], f32r, tag="w1")
            w2_sb = cpool.tile([128, 8 * 512], f32r, tag="w2")
            b0_sb = cpool.tile([128, 8], f32, tag="b0")
            xt_sb = cpool.tile([128, 4 * BCORE], f32, tag="xt")

            nc.sync.dma_start(
                out=w0h_sb[:].rearrange("p (k m) -> p k m", k=4),
                in_=W0h.rearrange("(k p) m -> p k m", p=128))
            nc.sync.dma_start(
                out=w0l_sb[:].rearrange("p (k m) -> p k m", k=4),
                in_=W0l.rearrange("(k p) m -> p k m", p=128))
            nc.sync.dma_start(
                out=w1_sb[:].rearrange("p (k m) -> p k m", k=8),
                in_=W1d.rearrange("(k p) m -> p k m", p=128))
            nc.sync.dma_start(
                out=w2_sb[:].rearrange("p (k m) -> p k m", k=8),
                in_=W2d.rearrange("(k p) m -> p k m", p=128))
            nc.sync.dma_start(out=b0_sb[:], in_=b0d)
            nc.sync.dma_start(
                out=xt_sb[:].rearrange("p (c b) -> p c b", c=4),
                in_=xT.rearrange("(c p) b -> p c b", p=128))

            # ---- persistent state ----
            negm0 = cpool.tile([128, 256], f32, tag="negm0")
            y1 = cpool.tile([128, 256], f32, tag="y1")
            negm1 = cpool.tile([128, 256], f32, tag="negm1")
            abar = cpool.tile([128, 256], f32, tag="abar")
            for st in (negm0, y1, negm1, abar):
                nc.vector.memset(st[:], 0.0)

            def stt(eng, out, in0, scalar, in1, op0, op1):
                eng.scalar_tensor_tensor(out=out, in0=in0, scalar=float(scalar),
                                         in1=in1, op0=op0, op1=op1)

            t0 = 0
            for blk, Tb in enumerate(BLOCK_SIZES):
                Nk = Tb * BCORE

                # -- spike generation: DMA R (fp32), compare into f32r spikes --
                sblk = s_pool.tile([128, 4 * Nk], f32r, tag="sblk")
                rt = rt_pool.tile([128, 4 * Nk], f32, tag="rt")
                rt4 = RT.rearrange("(c p) n -> p c n", p=128)
                for c in range(4):
                    nc.sync.dma_start(
                        out=rt[:, c * Nk:(c + 1) * Nk],
                        in_=rt4[:, c, t0 * BCORE: t0 * BCORE + Nk])
                for c in range(4):
                    xc = (xt_sb[:].rearrange("p (c b) -> p c b", c=4)[:, c]
                          .unsqueeze(1).broadcast_to([128, Tb, BCORE]))
                    ssl = sblk[:, c * Nk:(c + 1) * Nk].rearrange(
                        "p (t b) -> p t b", t=Tb)
                    rsl = rt[:, c * Nk:(c + 1) * Nk].rearrange(
                        "p (t b) -> p t b", t=Tb)
                    nc.vector.tensor_tensor(out=ssl, in0=xc, in1=rsl, op=A.is_gt)

                # -- matmul 0: H0 = S @ (W0h + W0l), feature-major --
                h0 = h0_pool.tile([128, 8 * Nk], f32, tag="h0")  # (t, c, b)
                h0v = h0[:].rearrange("p (t c b) -> p t c b", t=Tb, c=8)
                for c in range(8):
                    ps = ps_pool.tile([128, Nk], f32, tag="ps")
                    for ki in range(4):
                        nc.tensor.matmul(
                            ps[:],
                            lhsT=w0h_sb[:, ki * 1024 + c * 128: ki * 1024 + (c + 1) * 128],
                            rhs=sblk[:, ki * Nk:(ki + 1) * Nk],
                            start=(ki == 0), stop=False)
                    for ki in range(4):
                        nc.tensor.matmul(
                            ps[:],
                            lhsT=w0l_sb[:, ki * 1024 + c * 128: ki * 1024 + (c + 1) * 128],
                            rhs=sblk[:, ki * Nk:(ki + 1) * Nk],
                            start=False, stop=(ki == 3))
                    # PSUM -> SBUF with bias fold (always-spiking bias feature)
                    nc.scalar.activation(
                        out=h0v[:, :, c, :],
                        in_=ps[:].rearrange("p (t b) -> p t b", t=Tb),
                        func=Act.Identity, bias=b0_sb[:, c:c + 1], scale=1.0)

                # -- layer-0 recurrence --
                s0blk = s0_pool.tile([128, Tb * 256], f32r, tag="s0")  # (t, c, b)
                for t in range(Tb):
                    h0t = h0[:, t * 256:(t + 1) * 256]
                    m0t = tmp_pool.tile([128, 256], f32, tag="m0t")
                    stt(nc.vector, m0t[:], negm0[:], -BETA, h0t, A.mult, A.add)
                    s0sl = s0blk[:, t * 256:(t + 1) * 256]
                    nc.vector.tensor_scalar(out=s0sl, in0=m0t[:], scalar1=THR,
                                            scalar2=None, op0=A.is_gt)
                    stt(nc.vector, negm0[:], s0sl, 1.0, m0t[:], A.subtract, A.mult)

                # -- matmul 1: H1 = s0 @ W1, feature-major --
                h1 = h1_pool.tile([128, 8 * Nk], f32, tag="h1")  # (t, c, b)
                h1v = h1[:].rearrange("p (t c b) -> p t c b", t=Tb, c=8)
                s0v = s0blk[:].rearrange("p (t c b) -> p c t b", t=Tb, c=8)
                for c in range(8):
                    ps = ps_pool.tile([128, Nk], f32, tag="ps")
                    for ki in range(8):
                        nc.tensor.matmul(
                            ps[:],
                            lhsT=w1_sb[:, ki * 1024 + c * 128: ki * 1024 + (c + 1) * 128],
                            rhs=s0v[:, ki],
                            start=(ki == 0), stop=(ki == 7))
                    nc.scalar.activation(
                        out=h1v[:, :, c, :],
                        in_=ps[:].rearrange("p (t b) -> p t b", t=Tb),
                        func=Act.Copy)

                # -- layer-1 recurrence + weighted spike accumulation --
                for t in range(Tb):
                    h1t = h1[:, t * 256:(t + 1) * 256]
                    stt(nc.vector, y1[:], y1[:], ALPHA, h1t, A.mult, A.add)
                    m1t = tmp_pool.tile([128, 256], f32, tag="m1t")
                    stt(nc.vector, m1t[:], negm1[:], -BETA, y1[:], A.mult, A.add)
                    s1 = tmp_pool.tile([128, 256], f32, tag="s1")
                    nc.vector.tensor_scalar(out=s1[:], in0=m1t[:], scalar1=THR,
                                            scalar2=None, op0=A.is_gt)
                    stt(nc.vector, abar[:], s1[:], W[t0 + t], abar[:], A.mult, A.add)
                    stt(nc.vector, negm1[:], s1[:], 1.0, m1t[:], A.subtract, A.mult)

                t0 += Tb

            # ---- final: mem2 = A @ W2 ----
            af = cpool.tile([128, 256], f32r, tag="af")
            nc.vector.tensor_copy(af[:], abar[:])
            psf = ps_pool.tile([BCORE, 512], f32, tag="ps")
            for ki in range(8):
                nc.tensor.matmul(
                    psf[:],
                    lhsT=af[:, ki * BCORE:(ki + 1) * BCORE],
                    rhs=w2_sb[:, ki * 512:(ki + 1) * 512],
                    start=(ki == 0), stop=(ki == 7))
            outsb = cpool.tile([BCORE, 512], f32, tag="outsb")
            nc.scalar.activation(out=outsb[:], in_=psf[:], func=Act.Copy)
            nc.sync.dma_start(out=outd, in_=outsb[:])

    nc.compile()
    _CACHE["nc"] = nc
    return nc


def make_in_maps(inputs, W0, W1, W2, random_distribution):
    """Host-side shard prep: slice batch, transpose to feature-major,
    split/round weights for the FP22 matmul paths."""
    inputs = np.ascontiguousarray(np.asarray(inputs, np.float32))
    W0 = np.asarray(W0, np.float32)
    W1 = np.asarray(W1, np.float32)
    W2 = np.asarray(W2, np.float32)
    R = np.asarray(random_distribution, np.float32)

    W0h = np.ascontiguousarray(_round_m11(W0[:512]))
    W0l = np.ascontiguousarray(W0[:512] - W0h)   # hw rounds again -> exact split
    W1r = np.ascontiguousarray(_round_m11(W1))
    W2r = np.ascontiguousarray(_round_m11(W2))
    b0 = np.ascontiguousarray(W0[512].reshape(8, 128).T)  # [128, 8]

    in_maps = []
    for i in range(NCORES):
        sl = slice(i * BCORE, (i + 1) * BCORE)
        xTi = np.ascontiguousarray(inputs[sl].T)  # [512, 32]
        # [99, 32, 512] -> [512, 99*32] feature-major
        RTi = np.ascontiguousarray(
            R[1:, sl, :512].transpose(2, 0, 1).reshape(512, T * BCORE))
        in_maps.append({
            "RT": RTi, "xT": xTi, "W0h": W0h, "W0l": W0l,
            "W1d": W1r, "W2d": W2r, "b0d": b0,
        })
    return in_maps


def kernel(inputs, W0, W1, W2, random_distribution):
    from concourse.bass_utils import run_bass_kernel_spmd
    nc = build_program()
    in_maps = make_in_maps(inputs, W0, W1, W2, random_distribution)
    res = run_bass_kernel_spmd(nc, in_maps, core_ids=list(range(NCORES)))
    outs = [np.asarray(res.results[i]["out"], np.float32) for i in range(NCORES)]
    return np.concatenate(outs, axis=0)


if __name__ == "__main__":
    d = np.load("/tmp/snn_inputs.npz")
    out = kernel(d["inputs"], d["W0"], d["W1"], d["W2"], d["random_distribution"])
    exp = d["expected"]
    rel = np.linalg.norm(out - exp) / np.linalg.norm(exp)
    print("kernel vs reference rel_l2:", rel)
